# revision 2
# baseline (speedup 1.0000x reference)
"""CSPN 3x3 per-pixel MAC kernel for Trainium2, 8-core data parallel.

out[b,0,h,w] = sum_{t in 0..8, t!=4} K[b,t,h,w] * xpad[b,h+t//3,w+t%3]
             + K[b,4,h,w] * input0[b,0,h,w]

Sharding: batch 16 -> 2 samples per core, pure data parallel.

Structure (all choices A/B-measured on this silicon):
 - Host-side row-shifted weights + shifted-identity PE accumulation:
   weight plane t is shifted down by t//3 rows so the DVE multiplies
   row-ALIGNED operands against a SINGLE x load per band; the tap's row
   offset is recovered in the PE reduction matmul whose lhsT is a
   shifted identity (ps[m] += prod[m + t//3]).  This removes the 3x
   row-shifted x re-read of the original kernel (x DMA 5.1 -> 1.7 MB).
 - Bands of 128 padded rows overlap by 2 (126 output rows/band, 6
   bands per core); the band's products live on 128 partitions.
 - Mixed precision tuned to DVE perf modes (measured: tensor_tensor is
   604ns/op when ALL operands are bf16 [2x mode], 1374ns with any fp8
   operand [1x], 1826ns for fp8xfp8): the center pair (K4, x0) is bf16
   (2x mode), the 8 spatial weight planes stay fp8 e3m4 (1B DMA).
 - Engines: DVE 9 products; PE accumulates into f32 PSUM via the
   shifted identities (3 PSUM-bank column chunks); ACT copies
   PSUM->bf16 SBUF and issues the x/kf8 DMAs; SP issues kb16/out.

Measured negatives (do not retry): promoting ANY spatial tap to bf16
(>2 bf16 planes is a cliff: +10-20us); ACT-side fp8->bf16 upconversion
(ACT writes contend+serialize: +6-14us); fp8 products (NaN from e3m4
overflow + slower); fused multi-plane DVE APs (no gain); 64-row bands
(DVE cost doubles: +69us); kfpool bufs=3 (+12us); moving x or kb16
between DMA rings (+2-6us).  Relative error 1.297e-2 (gate 2e-2),
deterministic for the harness's fixed inputs.
"""

import os
import sys

for _p in ("/opt/trn_rl_repo", "/root/.axon_site/_ro/trn_rl_repo"):
    if os.path.isdir(_p) and _p not in sys.path:
        sys.path.append(_p)

import ml_dtypes
import numpy as np

import concourse.bacc as bacc
import concourse.mybir as mybir
from concourse import bass_utils, tile

KS = 3
BS, H, W = 16, 352, 1216
NCORES = 8
SPC = BS // NCORES          # samples per core = 2
A = H + 2                   # padded row space: 354
WP = W + 2                  # padded width: 1218
BF16 = mybir.dt.bfloat16
F8 = mybir.dt.float8e3
F32 = mybir.dt.float32
NP_BF16 = ml_dtypes.bfloat16
NP_F8 = ml_dtypes.float8_e3m4
MULT = mybir.AluOpType.mult

F8_TAPS = [0, 1, 2, 3, 5, 6, 7, 8]          # kf8 plane order
KF_PLANE = {t: p for p, t in enumerate(F8_TAPS)}

# (a0, pa, po): band covers padded rows a0..a0+pa-1, produces output
# rows a0..a0+po-1 (po = pa - 2: out row r needs padded rows r..r+2)
BANDS = [(0, 128, 126), (126, 128, 126), (252, 102, 100)]
CHUNKS = [(0, 512), (512, 512), (1024, 192)]   # PSUM-bank col chunks


def _build_nc(loop_reps=1, body_reps=1):
    nc = bacc.Bacc(None)
    kb16 = nc.dram_tensor("kb16", [SPC, A, 2, W], BF16, kind="ExternalInput")
    kf8 = nc.dram_tensor("kf8", [SPC, A, 8, W], F8, kind="ExternalInput")
    xpad = nc.dram_tensor("xpad", [SPC, A, WP], BF16, kind="ExternalInput")
    ident = nc.dram_tensor("ident", [128, 3 * 128], BF16, kind="ExternalInput")
    out = nc.dram_tensor("out", [SPC, H, W], BF16, kind="ExternalOutput")

    with tile.TileContext(nc) as tc:
        with (
            tc.tile_pool(name="ipool", bufs=1) as ipool,
            tc.tile_pool(name="kbpool", bufs=2) as kbpool,
            tc.tile_pool(name="kfpool", bufs=2) as kfpool,
            tc.tile_pool(name="xpool", bufs=4) as xpool,
            tc.tile_pool(name="prpool", bufs=2) as prpool,
            tc.tile_pool(name="pspool", bufs=2, space="PSUM") as pspool,
            tc.tile_pool(name="opool", bufs=4) as opool,
        ):
            it = ipool.tile([128, 3 * 128], BF16, tag="ident")
            nc.sync.dma_start(out=it[:, :], in_=ident[:, :])

            def body():
                for b in range(SPC):
                    for a0, pa, po in BANDS:
                        kbt = kbpool.tile([128, 2, W], BF16, tag="kbt")
                        kft = kfpool.tile([128, 8, W], F8, tag="kft")
                        xt = xpool.tile([128, WP], BF16, tag="xt")
                        nc.scalar.dma_start(
                            out=xt[:pa, :], in_=xpad[b, a0 : a0 + pa, :]
                        )
                        nc.scalar.dma_start(
                            out=kft[:pa, :, :],
                            in_=kf8[b, a0 : a0 + pa, :, :],
                        )
                        nc.sync.dma_start(
                            out=kbt[:pa, :, :],
                            in_=kb16[b, a0 : a0 + pa, :, :],
                        )

                        prod = prpool.tile([128, 9, W], BF16, tag="prod")
                        ps = pspool.tile([128, W], F32, tag="ps")
                        ot = opool.tile([128, W], BF16, tag="ot")

                        for t in range(9):
                            i, j = t // 3, t % 3
                            if t == 4:
                                k_src = kbt[:pa, 0, :]
                                src = kbt[:pa, 1, :]
                                pidx = 0
                            else:
                                k_src = kft[:pa, KF_PLANE[t], :]
                                src = xt[:pa, j : j + W]
                                pidx = 1 + KF_PLANE[t]
                            nc.vector.tensor_tensor(
                                out=prod[:pa, pidx, :], in0=k_src,
                                in1=src, op=MULT,
                            )
                            for w0, wc in CHUNKS:
                                nc.tensor.matmul(
                                    out=ps[:po, w0 : w0 + wc],
                                    lhsT=it[:pa, 128 * i : 128 * i + po],
                                    rhs=prod[:pa, pidx, w0 : w0 + wc],
                                    start=(t == 0), stop=(t == 8),
                                )
                        nc.scalar.copy(out=ot[:po, :], in_=ps[:po, :])
                        nc.sync.dma_start(
                            out=out[b, a0 : a0 + po, :], in_=ot[:po, :]
                        )

            def rep_body():
                for _ in range(body_reps):
                    body()

            if loop_reps == 1:
                rep_body()
            else:
                with tc.For_i(0, loop_reps, 1):
                    rep_body()
    nc.finalize()
    return nc


_NC_CACHE = None


def _get_nc():
    global _NC_CACHE
    if _NC_CACHE is None:
        _NC_CACHE = _build_nc()
    return _NC_CACHE


def _make_in_maps(kernel_arr, input_arr, input0_arr):
    kernel_arr = np.asarray(kernel_arr, dtype=np.float32)
    inp = np.asarray(input_arr, dtype=np.float32)[:, 0]
    inp0 = np.asarray(input0_arr, dtype=np.float32)[:, 0]

    # weight plane t shifted down by t//3 rows (x0 by 1) so the DVE
    # multiplies row-aligned operands; the PE shifted-identity undoes it
    kb = np.zeros((BS, A, 2, W), dtype=NP_BF16)
    kb[:, 1 : 1 + H, 0, :] = kernel_arr[:, 4].astype(NP_BF16)
    kb[:, 1 : 1 + H, 1, :] = inp0.astype(NP_BF16)

    kf = np.zeros((BS, A, 8, W), dtype=NP_F8)
    for p, t in enumerate(F8_TAPS):
        i = t // 3
        kf[:, i : i + H, p, :] = kernel_arr[:, t].astype(NP_F8)

    xp = np.zeros((BS, A, WP), dtype=NP_BF16)
    xp[:, 1 : H + 1, 1 : W + 1] = inp.astype(NP_BF16)

    # ident[q, 128*i + m] = 1 iff q == m + i (shifted identities)
    ident = np.zeros((128, 3 * 128), dtype=NP_BF16)
    for i in range(3):
        for m in range(128 - i):
            ident[m + i, 128 * i + m] = 1

    in_maps = []
    for c in range(NCORES):
        s = slice(c * SPC, (c + 1) * SPC)
        in_maps.append(
            {
                "kb16": np.ascontiguousarray(kb[s]),
                "kf8": np.ascontiguousarray(kf[s]),
                "xpad": np.ascontiguousarray(xp[s]),
                "ident": ident,
            }
        )
    return in_maps


def _run(kernel_arr, input_arr, input0_arr, trace=False):
    in_maps = _make_in_maps(kernel_arr, input_arr, input0_arr)
    nc = _get_nc()
    res = bass_utils.run_bass_kernel_spmd(
        nc, in_maps, list(range(NCORES)), trace=trace
    )
    out = np.concatenate([res.results[c]["out"] for c in range(NCORES)], axis=0)
    out = out.astype(np.float32)
    return np.ascontiguousarray(out.reshape(BS, 1, H, W)), res


def kernel(kernel, input, input0):  # noqa: A002 - names fixed by harness
    out, _ = _run(kernel, input, input0, trace=False)
    return out


# revision 3
# speedup vs baseline: 1.1965x; 1.1965x over previous
"""CSPN 3x3 per-pixel MAC kernel for Trainium2, 8-core data parallel.

out[b,0,h,w] = sum_{t in 0..8, t!=4} K[b,t,h,w] * xpad[b,h+t//3,w+t%3]
             + K[b,4,h,w] * input0[b,0,h,w]

Sharding: batch 16 -> 2 samples per core, pure data parallel.

Structure (every choice interleave-A/B-measured on this silicon):
 - Host-side row-shifted weight planes + shifted-identity PE lhsT:
   plane t is shifted down t//3 rows so the DVE multiplies row-ALIGNED
   operands against a SINGLE x load per band; the row offset is
   recovered in the PE reduction matmul (ps[m] += prod[m + t//3]).
   Removes the 3x row-shifted x re-read (x DMA 5.1 -> 1.7 MB/core).
 - Bands of 128 padded rows overlapping by 2 (126 out rows per band).
 - DVE perf modes measured: tensor_tensor = 604ns/op [128,1216] when
   ALL operands bf16 (2x mode), 1374ns with any fp8 operand (1x),
   1826ns fp8xfp8.  So: center pair (K4, x0) + corner taps 0,2,6,8
   are bf16; edge taps 1,3,5,7 stay fp8 e3m4 (1B DMA).
 - CRITICAL layout fact: bf16 weight tensors/tiles must stay at 2
   planes each -- a single >=3-plane bf16 tile costs +10-20us (cause
   unknown, trace unavailable); three separate [A, 2, W] tensors
   (kb16, kbx1, kbx2) carry the 6 bf16 planes with no penalty.
 - Engines: DVE 9 products; PE accumulates into f32 PSUM via shifted
   identities (3 PSUM-bank column chunks); ACT copies PSUM->bf16 SBUF
   and issues x/kf8 DMAs; SP issues kb16/kbx1/kbx2/out.

Measured negatives (do not retry): any >=3-plane bf16 tile; ACT-side
fp8->bf16 upconversion (+4-11us, writes contend); fp8 products (NaN +
slower); fused multi-plane DVE APs (no gain); 64-row bands (+69us,
DVE scales with band count x free-dim); kfpool bufs=3 (+12us).
Rel err 9.473e-3 (gate 2e-2), deterministic for the fixed inputs.
"""

import os
import sys

for _p in ("/opt/trn_rl_repo", "/root/.axon_site/_ro/trn_rl_repo"):
    if os.path.isdir(_p) and _p not in sys.path:
        sys.path.append(_p)

import ml_dtypes
import numpy as np

import concourse.bacc as bacc
import concourse.mybir as mybir
from concourse import bass_utils, tile

KS = 3
BS, H, W = 16, 352, 1216
NCORES = 8
SPC = BS // NCORES
A = H + 2
WP = W + 2
BF16 = mybir.dt.bfloat16
F8 = mybir.dt.float8e3
F32 = mybir.dt.float32
NP_BF16 = ml_dtypes.bfloat16
NP_F8 = ml_dtypes.float8_e3m4
MULT = mybir.AluOpType.mult

XTRA = [0, 2, 6, 8]
F8_TAPS = [t for t in (0, 1, 2, 3, 5, 6, 7, 8) if t not in XTRA]
KF_PLANE = {t: p for p, t in enumerate(F8_TAPS)}
NF = len(F8_TAPS)
X1 = XTRA[:2]               # taps in kbx1
X2 = XTRA[2:]               # taps in kbx2

BANDS = [(0, 128, 126), (126, 128, 126), (252, 102, 100)]
CHUNKS = [(0, 512), (512, 512), (1024, 192)]


def _build_nc(loop_reps=1, body_reps=1):
    nc = bacc.Bacc(None)
    kb16 = nc.dram_tensor("kb16", [SPC, A, 2, W], BF16, kind="ExternalInput")
    kbx1 = nc.dram_tensor("kbx1", [SPC, A, 2, W], BF16, kind="ExternalInput")
    kbx2 = (nc.dram_tensor("kbx2", [SPC, A, 2, W], BF16,
                           kind="ExternalInput") if X2 else None)
    kf8 = nc.dram_tensor("kf8", [SPC, A, NF, W], F8, kind="ExternalInput")
    xpad = nc.dram_tensor("xpad", [SPC, A, WP], BF16, kind="ExternalInput")
    ident = nc.dram_tensor("ident", [128, 3 * 128], BF16, kind="ExternalInput")
    out = nc.dram_tensor("out", [SPC, H, W], BF16, kind="ExternalOutput")

    with tile.TileContext(nc) as tc:
        with (
            tc.tile_pool(name="ipool", bufs=1) as ipool,
            tc.tile_pool(name="kbpool", bufs=2) as kbpool,
            tc.tile_pool(name="kx1pool", bufs=2) as kx1pool,
            tc.tile_pool(name="kx2pool", bufs=2) as kx2pool,
            tc.tile_pool(name="kfpool", bufs=2) as kfpool,
            tc.tile_pool(name="xpool", bufs=4) as xpool,
            tc.tile_pool(name="prpool", bufs=2) as prpool,
            tc.tile_pool(name="pspool", bufs=2, space="PSUM") as pspool,
            tc.tile_pool(name="opool", bufs=4) as opool,
        ):
            it = ipool.tile([128, 3 * 128], BF16, tag="ident")
            nc.sync.dma_start(out=it[:, :], in_=ident[:, :])

            def body():
                for b in range(SPC):
                    for a0, pa, po in BANDS:
                        kbt = kbpool.tile([128, 2, W], BF16, tag="kbt")
                        kx1 = kx1pool.tile([128, 2, W], BF16, tag="kx1")
                        kx2 = (kx2pool.tile([128, 2, W], BF16, tag="kx2",
                                            name="kx2")
                               if X2 else None)
                        kft = kfpool.tile([128, NF, W], F8, tag="kft")
                        xt = xpool.tile([128, WP], BF16, tag="xt")
                        nc.scalar.dma_start(
                            out=xt[:pa, :], in_=xpad[b, a0 : a0 + pa, :]
                        )
                        nc.scalar.dma_start(
                            out=kft[:pa, :, :],
                            in_=kf8[b, a0 : a0 + pa, :, :],
                        )
                        nc.sync.dma_start(
                            out=kbt[:pa, :, :],
                            in_=kb16[b, a0 : a0 + pa, :, :],
                        )
                        nc.sync.dma_start(
                            out=kx1[:pa, :, :],
                            in_=kbx1[b, a0 : a0 + pa, :, :],
                        )
                        if X2:
                            nc.sync.dma_start(
                                out=kx2[:pa, :, :],
                                in_=kbx2[b, a0 : a0 + pa, :, :],
                            )

                        prod = prpool.tile([128, 9, W], BF16, tag="prod")
                        ps = pspool.tile([128, W], F32, tag="ps")
                        ot = opool.tile([128, W], BF16, tag="ot")

                        for n, t in enumerate(range(9)):
                            i, j = t // 3, t % 3
                            if t == 4:
                                k_src = kbt[:pa, 0, :]
                                src = kbt[:pa, 1, :]
                            else:
                                if t in X1:
                                    k_src = kx1[:pa, X1.index(t), :]
                                elif t in X2:
                                    k_src = kx2[:pa, X2.index(t), :]
                                else:
                                    k_src = kft[:pa, KF_PLANE[t], :]
                                src = xt[:pa, j : j + W]
                            nc.vector.tensor_tensor(
                                out=prod[:pa, n, :], in0=k_src,
                                in1=src, op=MULT,
                            )
                            for w0, wc in CHUNKS:
                                nc.tensor.matmul(
                                    out=ps[:po, w0 : w0 + wc],
                                    lhsT=it[:pa, 128 * i : 128 * i + po],
                                    rhs=prod[:pa, n, w0 : w0 + wc],
                                    start=(t == 0), stop=(t == 8),
                                )
                        nc.scalar.copy(out=ot[:po, :], in_=ps[:po, :])
                        nc.sync.dma_start(
                            out=out[b, a0 : a0 + po, :], in_=ot[:po, :]
                        )

            def rep_body():
                for _ in range(body_reps):
                    body()

            if loop_reps == 1:
                rep_body()
            else:
                with tc.For_i(0, loop_reps, 1):
                    rep_body()
    nc.finalize()
    return nc


_NC_CACHE = None


def _get_nc():
    global _NC_CACHE
    if _NC_CACHE is None:
        _NC_CACHE = _build_nc()
    return _NC_CACHE


def _make_in_maps(kernel_arr, input_arr, input0_arr):
    kernel_arr = np.asarray(kernel_arr, dtype=np.float32)
    inp = np.asarray(input_arr, dtype=np.float32)[:, 0]
    inp0 = np.asarray(input0_arr, dtype=np.float32)[:, 0]

    kb = np.zeros((BS, A, 2, W), dtype=NP_BF16)
    kb[:, 1 : 1 + H, 0, :] = kernel_arr[:, 4].astype(NP_BF16)
    kb[:, 1 : 1 + H, 1, :] = inp0.astype(NP_BF16)

    def xtra_tensor(taps):
        kx = np.zeros((BS, A, 2, W), dtype=NP_BF16)
        for p, t in enumerate(taps):
            i = t // 3
            kx[:, i : i + H, p, :] = kernel_arr[:, t].astype(NP_BF16)
        return kx

    kx1 = xtra_tensor(X1)
    kx2 = xtra_tensor(X2) if X2 else None

    kf = np.zeros((BS, A, NF, W), dtype=NP_F8)
    for p, t in enumerate(F8_TAPS):
        i = t // 3
        kf[:, i : i + H, p, :] = kernel_arr[:, t].astype(NP_F8)

    xp = np.zeros((BS, A, WP), dtype=NP_BF16)
    xp[:, 1 : H + 1, 1 : W + 1] = inp.astype(NP_BF16)

    ident = np.zeros((128, 3 * 128), dtype=NP_BF16)
    for i in range(3):
        for m in range(128 - i):
            ident[m + i, 128 * i + m] = 1

    in_maps = []
    for c in range(NCORES):
        s = slice(c * SPC, (c + 1) * SPC)
        m = {
            "kb16": np.ascontiguousarray(kb[s]),
            "kbx1": np.ascontiguousarray(kx1[s]),
            "kf8": np.ascontiguousarray(kf[s]),
            "xpad": np.ascontiguousarray(xp[s]),
            "ident": ident,
        }
        if X2:
            m["kbx2"] = np.ascontiguousarray(kx2[s])
        in_maps.append(m)
    return in_maps


def _run(kernel_arr, input_arr, input0_arr, trace=False):
    in_maps = _make_in_maps(kernel_arr, input_arr, input0_arr)
    nc = _get_nc()
    res = bass_utils.run_bass_kernel_spmd(
        nc, in_maps, list(range(NCORES)), trace=trace
    )
    out = np.concatenate([res.results[c]["out"] for c in range(NCORES)], axis=0)
    out = out.astype(np.float32)
    return np.ascontiguousarray(out.reshape(BS, 1, H, W)), res


def kernel(kernel, input, input0):  # noqa: A002
    out, _ = _run(kernel, input, input0, trace=False)
    return out
